# revision 13
# baseline (speedup 1.0000x reference)
"""NT-Xent loss kernel for Trainium2, data-parallel over 8 NeuronCores.

Reference computation (see problem): z = concat(z_i, z_j) [8192, 512] f32,
row-L2-normalize, sim = (zn @ zn.T) / 0.5, per-row logsumexp with the
self-diagonal masked out, pos = sim[r, (r+4096) % 8192],
loss = mean(lse - pos).

Sharding: each core owns a 1024-row slab of the similarity matrix and needs
all of z as columns.  The host hands each core z block-ROTATED so the SPMD
program is identical on every core: core c sees column blocks
[c, c+1, ..., c+7] (mod 8, blocks of 1024 rows).  Then on every core the
self-diagonal lives on the local diagonal of column block 0 and the positive
lives on the diagonal of column block 4 (columns 4096..5119).

Per-core device program:
  zt [512, 8192] f32  -- rotated z, transposed (host-prepped layout)
  1. sumsq per column via ones-matmul on PE (squares on DVE, bf16); every
     PSUM row holds the same 512 sums, so row 0 of four blocks is DMA-
     gathered into one compact [4, 512] tile and 1/norm is computed there
     on DVE only (reciprocal_approx_fast seed -> linear sqrt seed -> two
     Newton rsqrt steps; DVE cost scales with free size, so the compact
     layout makes this ~free, and ACT never runs Ln/Sqrt -- avoiding both
     the banned Rsqrt and ACT table-set thrash with the main Exp).
  2. invnorm rows are DMA-broadcast (step-0 partition AP) to [128, 512] and
     zt is normalized on DVE, cast to bf16 (fp32 matmul is 4x slower on the
     PE: 2 weight passes x 2 cycles/column; bf16 keeps loss rel err ~1e-5).
  3. sim slab = znT[0:1024].T @ znT as 8 n-groups x 8 m-tiles of
     [128, 1024] PSUM tiles (two 512-wide matmul groups per tile, K = 512
     in 4 chunks of 128).  n-group-major order so the tail column blocks
     (whose normalization finishes last) are consumed last.
  4. P = exp(2*cos - 2) on ACT with accum_out giving row partial sums.
     The constant max 2.0 (= 1/TEMP >= any sim entry) makes the logsumexp
     stable without a per-row max pass.
  5. The self-diagonal contributes exp(2*|zn_r|^2 - 2) = 1 + O(1e-3) to the
     row sum; it is removed by subtracting exactly 1.0 (error ~1e-6 rel).
  6. pos extracted from the PSUM block at columns 4096+m*128 via
     (psum * 2) (*) I with accum_out on DVE.
  7. out[p, m] = ln(S - 1) + 2 - pos for local row m*128 + p.
Host sums the 8x[128, 8] outputs and divides by 8192.
"""

from contextlib import ExitStack

import numpy as np

import concourse.bass as bass
import concourse.mybir as mybir
import concourse.tile as tile
from concourse import bacc
from concourse.bass_utils import run_bass_kernel_spmd

B = 4096          # rows per input
N = 2 * B         # 8192 total rows
D = 512           # feature dim
NCORES = 8
SLAB = N // NCORES   # 1024 rows per core
KC = D // 128        # 4 contraction chunks
NB = N // 512        # 16 column blocks of 512
NG = NB // 2         # 8 column groups of 1024 (one [128,1024] psum tile each)
GRP = 4              # column blocks per invnorm gather group
MT = SLAB // 128     # 8 m-tiles per core
TEMP = 0.5
INV_T = 1.0 / TEMP   # 2.0 == 1/temperature == stable logsumexp max

F32 = mybir.dt.float32
BF16 = mybir.dt.bfloat16
AF = mybir.ActivationFunctionType
ALU = mybir.AluOpType
AX = mybir.AxisListType

# Linear minimax seed for sqrt(v) with v = 1/sumsq; sumsq ~ chi^2(512), so
# v in [1/1024, 1/256] with margin.  Two Newton rsqrt steps (which use the
# exact sumsq) refine any ~2% seed to ~4e-6 relative.
_V_LO, _V_HI = 1.0 / 1024.0, 1.0 / 256.0


def _sqrt_seed_coeffs():
    # Equioscillating linear fit a*v + b ~ sqrt(v) on [lo, hi]:
    lo, hi = _V_LO, _V_HI
    a = (np.sqrt(hi) - np.sqrt(lo)) / (hi - lo)
    # Extremum of error at v* where a = 1/(2 sqrt(v*)) -> v* = 1/(4a^2)
    vs = 1.0 / (4.0 * a * a)
    # Split the error between endpoints and the interior extremum:
    b = (np.sqrt(lo) - a * lo + np.sqrt(vs) - a * vs) / 2.0
    return float(a), float(b)


SEED_A, SEED_B = _sqrt_seed_coeffs()


def build_kernel_body(ctx: ExitStack, tc: "tile.TileContext",
                      zt: bass.AP, ident: bass.AP, out: bass.AP) -> None:
    nc = tc.nc

    ztpool = ctx.enter_context(tc.tile_pool(name="ztraw", bufs=8))
    znpool = ctx.enter_context(tc.tile_pool(name="zn", bufs=NB))
    sqpool = ctx.enter_context(tc.tile_pool(name="sq", bufs=2))
    gpool = ctx.enter_context(tc.tile_pool(name="gath", bufs=2))
    bcpool = ctx.enter_context(tc.tile_pool(name="bcast", bufs=4))
    ppool = ctx.enter_context(tc.tile_pool(name="p", bufs=3))
    scrpool = ctx.enter_context(tc.tile_pool(name="scr", bufs=2))
    sppool = ctx.enter_context(tc.tile_pool(name="spart", bufs=MT))
    accpool = ctx.enter_context(tc.tile_pool(name="acc", bufs=1))
    cstpool = ctx.enter_context(tc.tile_pool(name="cst", bufs=1))
    psum_ss = ctx.enter_context(tc.tile_pool(name="pss", bufs=2, space="PSUM"))
    psum_mm = ctx.enter_context(tc.tile_pool(name="pmm", bufs=3, space="PSUM"))

    ones = cstpool.tile([128, 128], BF16, tag="ones")
    nc.vector.memset(ones[:, :], 1.0)
    ident_t = cstpool.tile([128, 128], F32, tag="ident")
    nc.sync.dma_start(ident_t[:, :], ident)
    # activation() requires AP biases (no const-AP registry in this context)
    bias0 = cstpool.tile([128, 1], F32, tag="b0")
    nc.vector.memset(bias0[:, :], 0.0)
    biasm2 = cstpool.tile([128, 1], F32, tag="bm2")
    nc.vector.memset(biasm2[:, :], -INV_T)

    # zt [512, 8192] viewed as [p=128, c=4, n=8192]
    zt_v = zt.rearrange("(c p) n -> p c n", p=128)

    # ---- phase A: load, sumsq, compact invnorm, normalize-to-bf16 ----------
    zntiles = []
    for g in range(NB // GRP):
        raws, gath = [], gpool.tile([GRP, 512], F32, tag="gath")
        for q in range(GRP):
            nb = g * GRP + q
            zt_t = ztpool.tile([128, KC * 512], F32, tag="ztraw")
            src = zt_v[:, :, nb * 512:(nb + 1) * 512]
            nc.sync.dma_start(zt_t[:].rearrange("p (c n) -> p c n", c=KC), src)
            raws.append(zt_t)

            sq = sqpool.tile([128, KC * 512], BF16, tag="sq")
            nc.vector.tensor_mul(sq[:, :], zt_t[:, :], zt_t[:, :])

            ss = psum_ss.tile([128, 512], F32, tag="pss")
            for c in range(KC):
                nc.tensor.matmul(ss[:, :], ones[:, :],
                                 sq[:, c * 512:(c + 1) * 512],
                                 start=(c == 0), stop=(c == KC - 1))
            # all 128 PSUM rows are identical; engines need partition-0-
            # aligned APs, so ACT-copy row 0 to SBUF (Copy is in every ACT
            # table set -- no table load), then DMA it to partition q of
            # the gather tile (DMA handles partition offsets).
            srow = sqpool.tile([1, 512], F32, tag="srow")
            nc.scalar.copy(srow[:, :], ss[0:1, :])
            nc.sync.dma_start(gath[q:q + 1, :], srow[:, :])

        # compact rsqrt chain on [GRP, 512] (DVE only)
        inv = gpool.tile([GRP, 512], F32, tag="inv")
        nc.vector.reciprocal_approx_fast(inv[:, :], gath[:, :])
        y = gpool.tile([GRP, 512], F32, tag="y")
        nc.vector.tensor_scalar(y[:, :], inv[:, :], SEED_A, SEED_B,
                                ALU.mult, ALU.add)
        for _ in range(2):
            t1 = gpool.tile([GRP, 512], F32, tag="t1")
            nc.vector.tensor_mul(t1[:, :], y[:, :], y[:, :])
            t2 = gpool.tile([GRP, 512], F32, tag="t2")
            nc.vector.tensor_mul(t2[:, :], t1[:, :], gath[:, :])
            t3 = gpool.tile([GRP, 512], F32, tag="t3")
            nc.vector.tensor_scalar(t3[:, :], t2[:, :], -0.5, 1.5,
                                    ALU.mult, ALU.add)
            y2 = gpool.tile([GRP, 512], F32, tag="y")
            nc.vector.tensor_mul(y2[:, :], y[:, :], t3[:, :])
            y = y2

        for q in range(GRP):
            nb = g * GRP + q
            stage = sqpool.tile([1, 512], F32, tag="stage")
            nc.sync.dma_start(stage[:, :], y[q:q + 1, :])
            bc = bcpool.tile([128, 512], F32, tag="bcast")
            nc.gpsimd.partition_broadcast(bc[:, :], stage[:, :])
            zn_t = znpool.tile([128, KC * 512], BF16, tag="zn")
            for c in range(KC):
                nc.vector.tensor_mul(zn_t[:, c * 512:(c + 1) * 512],
                                     raws[q][:, c * 512:(c + 1) * 512],
                                     bc[:, :])
            zntiles.append(zn_t)

    # ---- phase B: slab matmul + streaming exp-sum (n-group major) ----------
    s_all = accpool.tile([128, MT], F32, tag="s_all")
    pos_all = accpool.tile([128, MT], F32, tag="pos_all")
    sparts = [sppool.tile([128, NG], F32, tag="spart", name=f"spart{m}")
              for m in range(MT)]
    for ng in range(NG):
        for m in range(MT):
            mb, mo = divmod(m, 4)
            mo *= 128
            ps = psum_mm.tile([128, 1024], F32, tag="pmm")
            for half in range(2):
                nb = 2 * ng + half
                for c in range(KC):
                    nc.tensor.matmul(
                        ps[:, half * 512:(half + 1) * 512],
                        zntiles[mb][:, c * 512 + mo:c * 512 + mo + 128],
                        zntiles[nb][:, c * 512:(c + 1) * 512],
                        start=(c == 0), stop=(c == KC - 1))
            p_t = ppool.tile([128, 1024], F32, tag="p")
            nc.scalar.activation(p_t[:, :], ps[:, :], AF.Exp,
                                 bias=biasm2[:, :], scale=INV_T,
                                 accum_out=sparts[m][:, ng:ng + 1])
            if ng == (NB // 2 + mb) // 2:
                # positive pair: diagonal of the [128,128] sub-block at
                # column 4096 + m*128; pos = 2*cos, via (ps*2) (*) I
                off = ((NB // 2 + mb) % 2) * 512 + mo
                scr = scrpool.tile([128, 128], F32, tag="scr")
                nc.vector.scalar_tensor_tensor(
                    scr[:, :], ps[:, off:off + 128], INV_T, ident_t[:, :],
                    ALU.mult, ALU.mult, accum_out=pos_all[:, m:m + 1])

    # ---- phase C: lse - pos -------------------------------------------------
    for m in range(MT):
        nc.vector.reduce_sum(s_all[:, m:m + 1], sparts[m][:, :], axis=AX.X)
    s_corr = accpool.tile([128, MT], F32, tag="s_corr")
    nc.vector.tensor_scalar_add(s_corr[:, :], s_all[:, :], -1.0)
    l_all = accpool.tile([128, MT], F32, tag="l_all")
    nc.scalar.activation(l_all[:, :], s_corr[:, :], AF.Ln, bias=bias0[:, :])
    out_t = accpool.tile([128, MT], F32, tag="out_t")
    # (ln S + 2) - pos
    nc.vector.scalar_tensor_tensor(out_t[:, :], l_all[:, :], INV_T,
                                   pos_all[:, :], ALU.add, ALU.subtract)
    nc.sync.dma_start(out, out_t[:, :])


_NC_CACHE = None


def build_nc() -> "bass.Bass":
    global _NC_CACHE
    if _NC_CACHE is not None:
        return _NC_CACHE
    nc = bacc.Bacc("TRN2", target_bir_lowering=False, debug=False)
    zt = nc.dram_tensor("zt", [D, N], F32, kind="ExternalInput")
    ident = nc.dram_tensor("ident", [128, 128], F32, kind="ExternalInput")
    out = nc.dram_tensor("out", [128, MT], F32, kind="ExternalOutput")
    with tile.TileContext(nc) as tc:
        with ExitStack() as ctx:
            build_kernel_body(ctx, tc, zt.ap(), ident.ap(), out.ap())
    nc.compile()
    _NC_CACHE = nc
    return nc


def make_in_maps(z_i: np.ndarray, z_j: np.ndarray) -> list[dict]:
    z = np.concatenate([np.asarray(z_i), np.asarray(z_j)], axis=0)
    assert z.shape == (N, D) and z.dtype == np.float32
    ident = np.eye(128, dtype=np.float32)
    in_maps = []
    for c in range(NCORES):
        blocks = [z[SLAB * ((c + b) % NCORES): SLAB * ((c + b) % NCORES) + SLAB]
                  for b in range(NCORES)]
        zrot = np.concatenate(blocks, axis=0)          # [8192, 512]
        zt = np.ascontiguousarray(zrot.T)              # [512, 8192]
        in_maps.append({"zt": zt, "ident": ident})
    return in_maps


def reduce_outputs(results: list[dict]) -> np.ndarray:
    total = 0.0
    for c in range(NCORES):
        total += results[c]["out"].astype(np.float64).sum()
    return np.float32(total / N)


def kernel(z_i: np.ndarray, z_j: np.ndarray) -> np.ndarray:
    nc = build_nc()
    in_maps = make_in_maps(z_i, z_j)
    res = run_bass_kernel_spmd(nc, in_maps, list(range(NCORES)))
    return reduce_outputs(res.results)


# revision 15
# speedup vs baseline: 1.2151x; 1.2151x over previous
"""NT-Xent loss kernel for Trainium2, data-parallel over 8 NeuronCores.

Reference computation (see problem): z = concat(z_i, z_j) [8192, 512] f32,
row-L2-normalize, sim = (zn @ zn.T) / 0.5, per-row logsumexp with the
self-diagonal masked out, pos = sim[r, (r+4096) % 8192],
loss = mean(lse - pos).

Sharding: each core owns a 1024-row slab of the similarity matrix and needs
all of z as columns.  The host hands each core z block-ROTATED so the SPMD
program is identical on every core: core c sees column blocks
[c, c+1, ..., c+7] (mod 8, blocks of 1024 rows).  Then on every core the
self-diagonal lives on the local diagonal of column block 0 and the positive
lives on the diagonal of column block 4 (columns 4096..5119).

Per-core device program:
  zt [512, 8192] f32  -- rotated z, transposed (host-prepped layout)
  1. sumsq per column via ones-matmul on PE (squares on DVE, bf16); every
     PSUM row holds the same 512 sums, so row 0 of four blocks is DMA-
     gathered into one compact [4, 512] tile and 1/norm is computed there
     on DVE only (reciprocal_approx_fast seed -> linear sqrt seed -> two
     Newton rsqrt steps; DVE cost scales with free size, so the compact
     layout makes this ~free, and ACT never runs Ln/Sqrt -- avoiding both
     the banned Rsqrt and ACT table-set thrash with the main Exp).
  2. invnorm rows are DMA-broadcast (step-0 partition AP) to [128, 512] and
     zt is normalized on DVE, cast to bf16 (fp32 matmul is 4x slower on the
     PE: 2 weight passes x 2 cycles/column; bf16 keeps loss rel err ~1e-5).
  3. sim slab = znT[0:1024].T @ znT as 8 n-groups x 8 m-tiles of
     [128, 1024] PSUM tiles (two 512-wide matmul groups per tile, K = 512
     in 4 chunks of 128).  n-group-major order so the tail column blocks
     (whose normalization finishes last) are consumed last.
  4. P = exp(2*cos - 2) on ACT with accum_out giving row partial sums.
     The constant max 2.0 (= 1/TEMP >= any sim entry) makes the logsumexp
     stable without a per-row max pass.
  5. The self-diagonal contributes exp(2*|zn_r|^2 - 2) = 1 + O(1e-3) to the
     row sum; it is removed by subtracting exactly 1.0 (error ~1e-6 rel).
  6. pos extracted from the PSUM block at columns 4096+m*128 via
     (psum * 2) (*) I with accum_out on DVE.
  7. out[p, m] = ln(S - 1) + 2 - pos for local row m*128 + p.
Host sums the 8x[128, 8] outputs and divides by 8192.
"""

from contextlib import ExitStack

import numpy as np

import concourse.bass as bass
import concourse.mybir as mybir
import concourse.tile as tile
from concourse import bacc
from concourse.bass_utils import run_bass_kernel_spmd

B = 4096          # rows per input
N = 2 * B         # 8192 total rows
D = 512           # feature dim
NCORES = 8
SLAB = N // NCORES   # 1024 rows per core
KC = D // 128        # 4 contraction chunks
NB = N // 512        # 16 column blocks of 512
NG = NB // 2         # 8 column groups of 1024 (one [128,1024] psum tile each)
GRP = 4              # column blocks per invnorm gather group
MT = SLAB // 128     # 8 m-tiles per core
TEMP = 0.5
INV_T = 1.0 / TEMP   # 2.0 == 1/temperature == stable logsumexp max

F32 = mybir.dt.float32
BF16 = mybir.dt.bfloat16
AF = mybir.ActivationFunctionType
ALU = mybir.AluOpType
AX = mybir.AxisListType

# Linear minimax seed for sqrt(v) with v = 1/sumsq; sumsq ~ chi^2(512), so
# v in [1/1024, 1/256] with margin.  Two Newton rsqrt steps (which use the
# exact sumsq) refine any ~2% seed to ~4e-6 relative.
_V_LO, _V_HI = 1.0 / 1024.0, 1.0 / 256.0


def _sqrt_seed_coeffs():
    # Equioscillating linear fit a*v + b ~ sqrt(v) on [lo, hi]:
    lo, hi = _V_LO, _V_HI
    a = (np.sqrt(hi) - np.sqrt(lo)) / (hi - lo)
    # Extremum of error at v* where a = 1/(2 sqrt(v*)) -> v* = 1/(4a^2)
    vs = 1.0 / (4.0 * a * a)
    # Split the error between endpoints and the interior extremum:
    b = (np.sqrt(lo) - a * lo + np.sqrt(vs) - a * vs) / 2.0
    return float(a), float(b)


SEED_A, SEED_B = _sqrt_seed_coeffs()


def build_kernel_body(ctx: ExitStack, tc: "tile.TileContext",
                      zt: bass.AP, ident: bass.AP, out: bass.AP) -> None:
    nc = tc.nc

    ztpool = ctx.enter_context(tc.tile_pool(name="ztraw", bufs=3))
    zbpool = ctx.enter_context(tc.tile_pool(name="zbf", bufs=8))
    znpool = ctx.enter_context(tc.tile_pool(name="zn", bufs=NB))
    sqpool = ctx.enter_context(tc.tile_pool(name="sq", bufs=2))
    gpool = ctx.enter_context(tc.tile_pool(name="gath", bufs=2))
    bcpool = ctx.enter_context(tc.tile_pool(name="bcast", bufs=4))
    ppool = ctx.enter_context(tc.tile_pool(name="p", bufs=3))
    scrpool = ctx.enter_context(tc.tile_pool(name="scr", bufs=2))
    sppool = ctx.enter_context(tc.tile_pool(name="spart", bufs=MT))
    accpool = ctx.enter_context(tc.tile_pool(name="acc", bufs=1))
    cstpool = ctx.enter_context(tc.tile_pool(name="cst", bufs=1))
    psum_ss = ctx.enter_context(tc.tile_pool(name="pss", bufs=2, space="PSUM"))
    psum_mm = ctx.enter_context(tc.tile_pool(name="pmm", bufs=3, space="PSUM"))

    ones = cstpool.tile([128, 128], BF16, tag="ones")
    nc.vector.memset(ones[:, :], 1.0)
    ident_t = cstpool.tile([128, 128], F32, tag="ident")
    nc.sync.dma_start(ident_t[:, :], ident)
    # activation() requires AP biases (no const-AP registry in this context)
    bias0 = cstpool.tile([128, 1], F32, tag="b0")
    nc.vector.memset(bias0[:, :], 0.0)
    biasm2 = cstpool.tile([128, 1], F32, tag="bm2")
    nc.vector.memset(biasm2[:, :], -INV_T)

    # zt [512, 8192] viewed as [p=128, c=4, n=8192]
    zt_v = zt.rearrange("(c p) n -> p c n", p=128)

    # ---- phase A: load, sumsq, compact invnorm, normalize-to-bf16 ----------
    zntiles = []
    for g in range(NB // GRP):
        raws, gath = [], gpool.tile([GRP, 512], F32, tag="gath")
        for q in range(GRP):
            nb = g * GRP + q
            zt_t = ztpool.tile([128, KC * 512], F32, tag="ztraw")
            src = zt_v[:, :, nb * 512:(nb + 1) * 512]
            nc.sync.dma_start(zt_t[:].rearrange("p (c n) -> p c n", c=KC), src)

            # cast to bf16 once; all later elementwise work runs in the DVE
            # 2x bf16 mode (f32 ops are 1x and made DVE the pipeline gate)
            zb_t = zbpool.tile([128, KC * 512], BF16, tag="zbf")
            nc.vector.tensor_copy(zb_t[:, :], zt_t[:, :])
            raws.append(zb_t)

            sq = sqpool.tile([128, KC * 512], BF16, tag="sq")
            nc.vector.tensor_mul(sq[:, :], zb_t[:, :], zb_t[:, :])

            ss = psum_ss.tile([128, 512], F32, tag="pss")
            for c in range(KC):
                nc.tensor.matmul(ss[:, :], ones[:, :],
                                 sq[:, c * 512:(c + 1) * 512],
                                 start=(c == 0), stop=(c == KC - 1))
            # all 128 PSUM rows are identical; engines need partition-0-
            # aligned APs, so ACT-copy row 0 to SBUF (Copy is in every ACT
            # table set -- no table load), then DMA it to partition q of
            # the gather tile (DMA handles partition offsets).
            srow = sqpool.tile([1, 512], F32, tag="srow")
            nc.scalar.copy(srow[:, :], ss[0:1, :])
            nc.sync.dma_start(gath[q:q + 1, :], srow[:, :])

        # compact rsqrt chain on [GRP, 512] (DVE only)
        inv = gpool.tile([GRP, 512], F32, tag="inv")
        nc.vector.reciprocal_approx_fast(inv[:, :], gath[:, :])
        y = gpool.tile([GRP, 512], F32, tag="y")
        nc.vector.tensor_scalar(y[:, :], inv[:, :], SEED_A, SEED_B,
                                ALU.mult, ALU.add)
        for _ in range(2):
            t1 = gpool.tile([GRP, 512], F32, tag="t1")
            nc.vector.tensor_mul(t1[:, :], y[:, :], y[:, :])
            t2 = gpool.tile([GRP, 512], F32, tag="t2")
            nc.vector.tensor_mul(t2[:, :], t1[:, :], gath[:, :])
            t3 = gpool.tile([GRP, 512], F32, tag="t3")
            nc.vector.tensor_scalar(t3[:, :], t2[:, :], -0.5, 1.5,
                                    ALU.mult, ALU.add)
            y2 = gpool.tile([GRP, 512], F32, tag="y")
            nc.vector.tensor_mul(y2[:, :], y[:, :], t3[:, :])
            y = y2

        yb = gpool.tile([GRP, 512], BF16, tag="yb")
        nc.vector.tensor_copy(yb[:, :], y[:, :])
        for q in range(GRP):
            nb = g * GRP + q
            stage = sqpool.tile([1, 512], BF16, tag="stage")
            nc.sync.dma_start(stage[:, :], yb[q:q + 1, :])
            bc = bcpool.tile([128, 512], BF16, tag="bcast")
            nc.gpsimd.partition_broadcast(bc[:, :], stage[:, :])
            zn_t = znpool.tile([128, KC * 512], BF16, tag="zn")
            for c in range(KC):
                nc.vector.tensor_mul(zn_t[:, c * 512:(c + 1) * 512],
                                     raws[q][:, c * 512:(c + 1) * 512],
                                     bc[:, :])
            zntiles.append(zn_t)

    # ---- phase B: slab matmul + streaming exp-sum (n-group major) ----------
    s_all = accpool.tile([128, MT], F32, tag="s_all")
    pos_all = accpool.tile([128, MT], F32, tag="pos_all")
    sparts = [sppool.tile([128, NG], F32, tag="spart", name=f"spart{m}")
              for m in range(MT)]
    for ng in range(NG):
        for m in range(MT):
            mb, mo = divmod(m, 4)
            mo *= 128
            ps = psum_mm.tile([128, 1024], F32, tag="pmm")
            for half in range(2):
                nb = 2 * ng + half
                for c in range(KC):
                    nc.tensor.matmul(
                        ps[:, half * 512:(half + 1) * 512],
                        zntiles[mb][:, c * 512 + mo:c * 512 + mo + 128],
                        zntiles[nb][:, c * 512:(c + 1) * 512],
                        start=(c == 0), stop=(c == KC - 1))
            p_t = ppool.tile([128, 1024], F32, tag="p")
            nc.scalar.activation(p_t[:, :], ps[:, :], AF.Exp,
                                 bias=biasm2[:, :], scale=INV_T,
                                 accum_out=sparts[m][:, ng:ng + 1])
            if ng == (NB // 2 + mb) // 2:
                # positive pair: diagonal of the [128,128] sub-block at
                # column 4096 + m*128; pos = 2*cos, via (ps*2) (*) I
                off = ((NB // 2 + mb) % 2) * 512 + mo
                scr = scrpool.tile([128, 128], F32, tag="scr")
                nc.vector.scalar_tensor_tensor(
                    scr[:, :], ps[:, off:off + 128], INV_T, ident_t[:, :],
                    ALU.mult, ALU.mult, accum_out=pos_all[:, m:m + 1])

    # ---- phase C: lse - pos -------------------------------------------------
    for m in range(MT):
        nc.vector.reduce_sum(s_all[:, m:m + 1], sparts[m][:, :], axis=AX.X)
    s_corr = accpool.tile([128, MT], F32, tag="s_corr")
    nc.vector.tensor_scalar_add(s_corr[:, :], s_all[:, :], -1.0)
    l_all = accpool.tile([128, MT], F32, tag="l_all")
    nc.scalar.activation(l_all[:, :], s_corr[:, :], AF.Ln, bias=bias0[:, :])
    out_t = accpool.tile([128, MT], F32, tag="out_t")
    # (ln S + 2) - pos
    nc.vector.scalar_tensor_tensor(out_t[:, :], l_all[:, :], INV_T,
                                   pos_all[:, :], ALU.add, ALU.subtract)
    nc.sync.dma_start(out, out_t[:, :])


_NC_CACHE = None


def build_nc() -> "bass.Bass":
    global _NC_CACHE
    if _NC_CACHE is not None:
        return _NC_CACHE
    nc = bacc.Bacc("TRN2", target_bir_lowering=False, debug=False)
    zt = nc.dram_tensor("zt", [D, N], F32, kind="ExternalInput")
    ident = nc.dram_tensor("ident", [128, 128], F32, kind="ExternalInput")
    out = nc.dram_tensor("out", [128, MT], F32, kind="ExternalOutput")
    with tile.TileContext(nc) as tc:
        with ExitStack() as ctx:
            build_kernel_body(ctx, tc, zt.ap(), ident.ap(), out.ap())
    nc.compile()
    _NC_CACHE = nc
    return nc


def make_in_maps(z_i: np.ndarray, z_j: np.ndarray) -> list[dict]:
    z = np.concatenate([np.asarray(z_i), np.asarray(z_j)], axis=0)
    assert z.shape == (N, D) and z.dtype == np.float32
    ident = np.eye(128, dtype=np.float32)
    in_maps = []
    for c in range(NCORES):
        blocks = [z[SLAB * ((c + b) % NCORES): SLAB * ((c + b) % NCORES) + SLAB]
                  for b in range(NCORES)]
        zrot = np.concatenate(blocks, axis=0)          # [8192, 512]
        zt = np.ascontiguousarray(zrot.T)              # [512, 8192]
        in_maps.append({"zt": zt, "ident": ident})
    return in_maps


def reduce_outputs(results: list[dict]) -> np.ndarray:
    total = 0.0
    for c in range(NCORES):
        total += results[c]["out"].astype(np.float64).sum()
    return np.float32(total / N)


def kernel(z_i: np.ndarray, z_j: np.ndarray) -> np.ndarray:
    nc = build_nc()
    in_maps = make_in_maps(z_i, z_j)
    res = run_bass_kernel_spmd(nc, in_maps, list(range(NCORES)))
    return reduce_outputs(res.results)


# revision 16
# speedup vs baseline: 1.2838x; 1.0566x over previous
"""NT-Xent loss kernel for Trainium2, data-parallel over 8 NeuronCores.

Reference computation (see problem): z = concat(z_i, z_j) [8192, 512] f32,
row-L2-normalize, sim = (zn @ zn.T) / 0.5, per-row logsumexp with the
self-diagonal masked out, pos = sim[r, (r+4096) % 8192],
loss = mean(lse - pos).

Sharding: each core owns a 1024-row slab of the similarity matrix and needs
all of z as columns.  The host hands each core z block-ROTATED so the SPMD
program is identical on every core: core c sees column blocks
[c, c+1, ..., c+7] (mod 8, blocks of 1024 rows).  Then on every core the
self-diagonal lives on the local diagonal of column block 0 and the positive
lives on the diagonal of column block 4 (columns 4096..5119).

Per-core device program:
  zt [512, 8192] f32  -- rotated z, transposed (host-prepped layout)
  1. sumsq per column via ones-matmul on PE (squares on DVE, bf16); every
     PSUM row holds the same 512 sums, so row 0 of four blocks is DMA-
     gathered into one compact [4, 512] tile and 1/norm is computed there
     on DVE only (reciprocal_approx_fast seed -> linear sqrt seed -> two
     Newton rsqrt steps; DVE cost scales with free size, so the compact
     layout makes this ~free, and ACT never runs Ln/Sqrt -- avoiding both
     the banned Rsqrt and ACT table-set thrash with the main Exp).
  2. invnorm rows are DMA-broadcast (step-0 partition AP) to [128, 512] and
     zt is normalized on DVE, cast to bf16 (fp32 matmul is 4x slower on the
     PE: 2 weight passes x 2 cycles/column; bf16 keeps loss rel err ~1e-5).
  3. sim slab = znT[0:1024].T @ znT as 8 n-groups x 8 m-tiles of
     [128, 1024] PSUM tiles (two 512-wide matmul groups per tile, K = 512
     in 4 chunks of 128).  n-group-major order so the tail column blocks
     (whose normalization finishes last) are consumed last.
  4. P = exp(2*cos - 2) on ACT with accum_out giving row partial sums.
     The constant max 2.0 (= 1/TEMP >= any sim entry) makes the logsumexp
     stable without a per-row max pass.
  5. The self-diagonal contributes exp(2*|zn_r|^2 - 2) = 1 + O(1e-3) to the
     row sum; it is removed by subtracting exactly 1.0 (error ~1e-6 rel).
  6. pos extracted from the PSUM block at columns 4096+m*128 via
     (psum * 2) (*) I with accum_out on DVE.
  7. out[p, m] = ln(S - 1) + 2 - pos for local row m*128 + p.
Host sums the 8x[128, 8] outputs and divides by 8192.
"""

from contextlib import ExitStack

import numpy as np

import concourse.bass as bass
import concourse.mybir as mybir
import concourse.tile as tile
from concourse import bacc
from concourse.bass_utils import run_bass_kernel_spmd

B = 4096          # rows per input
N = 2 * B         # 8192 total rows
D = 512           # feature dim
NCORES = 8
SLAB = N // NCORES   # 1024 rows per core
KC = D // 128        # 4 contraction chunks
NB = N // 512        # 16 column blocks of 512
NG = NB // 2         # 8 column groups of 1024 (one [128,1024] psum tile each)
GRP = 2              # column blocks per invnorm gather group
MT = SLAB // 128     # 8 m-tiles per core
TEMP = 0.5
INV_T = 1.0 / TEMP   # 2.0 == 1/temperature == stable logsumexp max

F32 = mybir.dt.float32
BF16 = mybir.dt.bfloat16
AF = mybir.ActivationFunctionType
ALU = mybir.AluOpType
AX = mybir.AxisListType

# Linear minimax seed for sqrt(v) with v = 1/sumsq; sumsq ~ chi^2(512), so
# v in [1/1024, 1/256] with margin.  Two Newton rsqrt steps (which use the
# exact sumsq) refine any ~2% seed to ~4e-6 relative.
_V_LO, _V_HI = 1.0 / 1024.0, 1.0 / 256.0


def _sqrt_seed_coeffs():
    # Equioscillating linear fit a*v + b ~ sqrt(v) on [lo, hi]:
    lo, hi = _V_LO, _V_HI
    a = (np.sqrt(hi) - np.sqrt(lo)) / (hi - lo)
    # Extremum of error at v* where a = 1/(2 sqrt(v*)) -> v* = 1/(4a^2)
    vs = 1.0 / (4.0 * a * a)
    # Split the error between endpoints and the interior extremum:
    b = (np.sqrt(lo) - a * lo + np.sqrt(vs) - a * vs) / 2.0
    return float(a), float(b)


SEED_A, SEED_B = _sqrt_seed_coeffs()


def build_kernel_body(ctx: ExitStack, tc: "tile.TileContext",
                      zt: bass.AP, ident: bass.AP, out: bass.AP) -> None:
    nc = tc.nc

    ztpool = ctx.enter_context(tc.tile_pool(name="ztraw", bufs=4))
    zbpool = ctx.enter_context(tc.tile_pool(name="zbf", bufs=8))
    znpool = ctx.enter_context(tc.tile_pool(name="zn", bufs=NB))
    sqpool = ctx.enter_context(tc.tile_pool(name="sq", bufs=2))
    gpool = ctx.enter_context(tc.tile_pool(name="gath", bufs=2))
    bcpool = ctx.enter_context(tc.tile_pool(name="bcast", bufs=4))
    ppool = ctx.enter_context(tc.tile_pool(name="p", bufs=3))
    scrpool = ctx.enter_context(tc.tile_pool(name="scr", bufs=2))
    sppool = ctx.enter_context(tc.tile_pool(name="spart", bufs=MT))
    accpool = ctx.enter_context(tc.tile_pool(name="acc", bufs=1))
    cstpool = ctx.enter_context(tc.tile_pool(name="cst", bufs=1))
    psum_ss = ctx.enter_context(tc.tile_pool(name="pss", bufs=2, space="PSUM"))
    psum_mm = ctx.enter_context(tc.tile_pool(name="pmm", bufs=3, space="PSUM"))

    ones = cstpool.tile([128, 128], BF16, tag="ones")
    nc.vector.memset(ones[:, :], 1.0)
    ident_t = cstpool.tile([128, 128], F32, tag="ident")
    nc.sync.dma_start(ident_t[:, :], ident)
    # activation() requires AP biases (no const-AP registry in this context)
    bias0 = cstpool.tile([128, 1], F32, tag="b0")
    nc.vector.memset(bias0[:, :], 0.0)
    biasm2 = cstpool.tile([128, 1], F32, tag="bm2")
    nc.vector.memset(biasm2[:, :], -INV_T)

    # zt [512, 8192] viewed as [p=128, c=4, n=8192]
    zt_v = zt.rearrange("(c p) n -> p c n", p=128)

    # ---- phase A: load, sumsq, compact invnorm, normalize-to-bf16 ----------
    zntiles = []
    for g in range(NB // GRP):
        raws, gath = [], gpool.tile([GRP, 512], F32, tag="gath")
        for q in range(GRP):
            nb = g * GRP + q
            zt_t = ztpool.tile([128, KC * 512], F32, tag="ztraw")
            src = zt_v[:, :, nb * 512:(nb + 1) * 512]
            nc.sync.dma_start(zt_t[:].rearrange("p (c n) -> p c n", c=KC), src)

            # cast to bf16 once; all later elementwise work runs in the DVE
            # 2x bf16 mode (f32 ops are 1x and made DVE the pipeline gate)
            zb_t = zbpool.tile([128, KC * 512], BF16, tag="zbf")
            nc.vector.tensor_copy(zb_t[:, :], zt_t[:, :])
            raws.append(zb_t)

            sq = sqpool.tile([128, KC * 512], BF16, tag="sq")
            nc.vector.tensor_mul(sq[:, :], zb_t[:, :], zb_t[:, :])

            ss = psum_ss.tile([128, 512], F32, tag="pss")
            for c in range(KC):
                nc.tensor.matmul(ss[:, :], ones[:, :],
                                 sq[:, c * 512:(c + 1) * 512],
                                 start=(c == 0), stop=(c == KC - 1))
            # all 128 PSUM rows are identical; engines need partition-0-
            # aligned APs, so ACT-copy row 0 to SBUF (Copy is in every ACT
            # table set -- no table load), then DMA it to partition q of
            # the gather tile (DMA handles partition offsets).
            srow = sqpool.tile([1, 512], F32, tag="srow")
            nc.scalar.copy(srow[:, :], ss[0:1, :])
            nc.sync.dma_start(gath[q:q + 1, :], srow[:, :])

        # compact rsqrt chain on [GRP, 512] (DVE only)
        inv = gpool.tile([GRP, 512], F32, tag="inv")
        nc.vector.reciprocal_approx_fast(inv[:, :], gath[:, :])
        y = gpool.tile([GRP, 512], F32, tag="y")
        nc.vector.tensor_scalar(y[:, :], inv[:, :], SEED_A, SEED_B,
                                ALU.mult, ALU.add)
        for _ in range(2):
            t1 = gpool.tile([GRP, 512], F32, tag="t1")
            nc.vector.tensor_mul(t1[:, :], y[:, :], y[:, :])
            t2 = gpool.tile([GRP, 512], F32, tag="t2")
            nc.vector.tensor_mul(t2[:, :], t1[:, :], gath[:, :])
            t3 = gpool.tile([GRP, 512], F32, tag="t3")
            nc.vector.tensor_scalar(t3[:, :], t2[:, :], -0.5, 1.5,
                                    ALU.mult, ALU.add)
            y2 = gpool.tile([GRP, 512], F32, tag="y")
            nc.vector.tensor_mul(y2[:, :], y[:, :], t3[:, :])
            y = y2

        yb = gpool.tile([GRP, 512], BF16, tag="yb")
        nc.vector.tensor_copy(yb[:, :], y[:, :])
        for q in range(GRP):
            nb = g * GRP + q
            stage = sqpool.tile([1, 512], BF16, tag="stage")
            nc.sync.dma_start(stage[:, :], yb[q:q + 1, :])
            bc = bcpool.tile([128, 512], BF16, tag="bcast")
            nc.gpsimd.partition_broadcast(bc[:, :], stage[:, :])
            zn_t = znpool.tile([128, KC * 512], BF16, tag="zn")
            for c in range(KC):
                nc.vector.tensor_mul(zn_t[:, c * 512:(c + 1) * 512],
                                     raws[q][:, c * 512:(c + 1) * 512],
                                     bc[:, :])
            zntiles.append(zn_t)

    # ---- phase B: slab matmul + streaming exp-sum (n-group major) ----------
    s_all = accpool.tile([128, MT], F32, tag="s_all")
    pos_all = accpool.tile([128, MT], F32, tag="pos_all")
    sparts = [sppool.tile([128, NG], F32, tag="spart", name=f"spart{m}")
              for m in range(MT)]
    for ng in range(NG):
        for m in range(MT):
            mb, mo = divmod(m, 4)
            mo *= 128
            ps = psum_mm.tile([128, 1024], F32, tag="pmm")
            for half in range(2):
                nb = 2 * ng + half
                for c in range(KC):
                    nc.tensor.matmul(
                        ps[:, half * 512:(half + 1) * 512],
                        zntiles[mb][:, c * 512 + mo:c * 512 + mo + 128],
                        zntiles[nb][:, c * 512:(c + 1) * 512],
                        start=(c == 0), stop=(c == KC - 1))
            p_t = ppool.tile([128, 1024], F32, tag="p")
            nc.scalar.activation(p_t[:, :], ps[:, :], AF.Exp,
                                 bias=biasm2[:, :], scale=INV_T,
                                 accum_out=sparts[m][:, ng:ng + 1])
            if ng == (NB // 2 + mb) // 2:
                # positive pair: diagonal of the [128,128] sub-block at
                # column 4096 + m*128; pos = 2*cos, via (ps*2) (*) I
                off = ((NB // 2 + mb) % 2) * 512 + mo
                scr = scrpool.tile([128, 128], F32, tag="scr")
                nc.vector.scalar_tensor_tensor(
                    scr[:, :], ps[:, off:off + 128], INV_T, ident_t[:, :],
                    ALU.mult, ALU.mult, accum_out=pos_all[:, m:m + 1])

    # ---- phase C: lse - pos -------------------------------------------------
    for m in range(MT):
        nc.vector.reduce_sum(s_all[:, m:m + 1], sparts[m][:, :], axis=AX.X)
    s_corr = accpool.tile([128, MT], F32, tag="s_corr")
    nc.vector.tensor_scalar_add(s_corr[:, :], s_all[:, :], -1.0)
    l_all = accpool.tile([128, MT], F32, tag="l_all")
    nc.scalar.activation(l_all[:, :], s_corr[:, :], AF.Ln, bias=bias0[:, :])
    out_t = accpool.tile([128, MT], F32, tag="out_t")
    # (ln S + 2) - pos
    nc.vector.scalar_tensor_tensor(out_t[:, :], l_all[:, :], INV_T,
                                   pos_all[:, :], ALU.add, ALU.subtract)
    nc.sync.dma_start(out, out_t[:, :])


_NC_CACHE = None


def build_nc() -> "bass.Bass":
    global _NC_CACHE
    if _NC_CACHE is not None:
        return _NC_CACHE
    nc = bacc.Bacc("TRN2", target_bir_lowering=False, debug=False)
    zt = nc.dram_tensor("zt", [D, N], F32, kind="ExternalInput")
    ident = nc.dram_tensor("ident", [128, 128], F32, kind="ExternalInput")
    out = nc.dram_tensor("out", [128, MT], F32, kind="ExternalOutput")
    with tile.TileContext(nc) as tc:
        with ExitStack() as ctx:
            build_kernel_body(ctx, tc, zt.ap(), ident.ap(), out.ap())
    nc.compile()
    _NC_CACHE = nc
    return nc


def make_in_maps(z_i: np.ndarray, z_j: np.ndarray) -> list[dict]:
    z = np.concatenate([np.asarray(z_i), np.asarray(z_j)], axis=0)
    assert z.shape == (N, D) and z.dtype == np.float32
    ident = np.eye(128, dtype=np.float32)
    in_maps = []
    for c in range(NCORES):
        blocks = [z[SLAB * ((c + b) % NCORES): SLAB * ((c + b) % NCORES) + SLAB]
                  for b in range(NCORES)]
        zrot = np.concatenate(blocks, axis=0)          # [8192, 512]
        zt = np.ascontiguousarray(zrot.T)              # [512, 8192]
        in_maps.append({"zt": zt, "ident": ident})
    return in_maps


def reduce_outputs(results: list[dict]) -> np.ndarray:
    total = 0.0
    for c in range(NCORES):
        total += results[c]["out"].astype(np.float64).sum()
    return np.float32(total / N)


def kernel(z_i: np.ndarray, z_j: np.ndarray) -> np.ndarray:
    nc = build_nc()
    in_maps = make_in_maps(z_i, z_j)
    res = run_bass_kernel_spmd(nc, in_maps, list(range(NCORES)))
    return reduce_outputs(res.results)
